# Initial kernel scaffold
#
"""Bass/Trainium2 kernel for DecodeMultiHeadAttention (16 heads, B=2, T=1024, C=1024).

Sharding: tensor-parallel over heads -- 2 heads per core x 8 cores.

Math notes (vs the jax reference):
  * The ALiBi bias is base**clip(j-i,0) which is exactly 1.0 on every causal
    (unmasked) position, and softmax is shift-invariant, so it drops out.
  * Scores are tiny (std ~0.1) so softmax needs no max-subtraction:
    wei = exp(s*scale) / sum(exp(s*scale)) over the causal extent.

Per core (2 local heads h in {0,1}, batch b in {0,1}):
  * qT,kT = [d,t]-layout projections (PE contracts embedding dim c, using a
    host-transposed xT input so all DMA is contiguous).
  * v in natural [t,d] layout with a ones-column appended, so a single PE
    matmul per pair produces both (p @ v)^T and the softmax denominator.
  * Scores are computed transposed, sT[s,t] = k_s . q_t, so the p @ v matmul
    needs no on-chip transpose. wei is written transposed ([s,t]) and the
    host swaps the last two axes while unsharding.
  * Only the causal (lower-triangle in [t,s] == upper in [s,t]) block rows are
    computed and written; the rest of the wei output buffer stays zero
    (outputs are zero-initialized).
  * Output projection: each core computes a partial product over its 128 head
    dims (+ bias/8); the host sums the 8 partials.
"""

import numpy as np

import concourse.bacc as bacc
import concourse.tile as tile
from concourse import mybir
from concourse import bass_utils

N_CORES = 8
B, T, C = 2, 1024, 1024
H, D = 16, 64          # total heads, head dim
HL = H // N_CORES      # heads per core (2)
NT = B * T             # 2048 rows total
P = 128
KC = C // P            # 8 contraction tiles over embedding dim
TT = NT // P           # 16 row tiles
SM = T // P            # 8 s-tiles per (head, batch) pair
SCALE = float(C) ** -0.5
F32 = mybir.dt.float32
BF16 = mybir.dt.bfloat16

_CACHED = {}


def _body(tc, xT, wq, wk, wv, projT, bias8, weiT, partial):
    nc = tc.nc
    AX = mybir.AxisListType
    Exp = mybir.ActivationFunctionType.Exp

    const = tc.tile_pool(name="const", bufs=1)
    stage = tc.tile_pool(name="stage", bufs=2)
    ppool = tc.tile_pool(name="ppool", bufs=2)
    wpool = tc.tile_pool(name="wpool", bufs=4)
    rpool = tc.tile_pool(name="rpool", bufs=2)
    opool = tc.tile_pool(name="opool", bufs=4)
    psum = tc.tile_pool(name="psum", bufs=3, space="PSUM")
    psum_oa = tc.tile_pool(name="psum_oa", bufs=2, space="PSUM")

    # ---- persistent SBUF tensors ----
    xbf = const.tile([P, KC, NT], BF16)       # x^T, bf16: [c-part, c-tile, t]
    wbf = const.tile([P, 3, KC, P], BF16)     # q/k/v weights: [c-part, proj, c-tile, d]
    qTb = const.tile([P, NT], BF16)           # q^T: [2 heads x 64 d, t]
    kTb = const.tile([P, NT], BF16)
    v_sb = const.tile([P, TT, 2 * (D + 1)], BF16)  # v natural + ones cols
    pj0 = const.tile([D, C], BF16)            # projT rows 0:64   (local head 0)
    pj1 = const.tile([D, C], BF16)            # projT rows 64:128 (local head 1)
    hoT0 = const.tile([D, NT], BF16)          # head-out^T, local head 0
    hoT1 = const.tile([D, NT], BF16)
    bias_bc = const.tile([P, C], F32)         # bias/8 broadcast to 128 partitions

    # ---- load + cast inputs ----
    for k in range(KC):
        xf = stage.tile([P, NT], F32, tag="xstage")
        nc.sync.dma_start(out=xf, in_=xT[k * P:(k + 1) * P, :])
        nc.scalar.copy(out=xbf[:, k, :], in_=xf)

    for i, w in enumerate([wq, wk, wv]):
        wf = stage.tile([P, KC, P], F32, tag="wstage")
        nc.sync.dma_start(out=wf, in_=w.rearrange("(k p) d -> p k d", p=P))
        nc.vector.tensor_copy(out=wbf[:, i], in_=wf)

    pjf = stage.tile([P, C], F32, tag="pjstage")
    nc.sync.dma_start(out=pjf, in_=projT)
    nc.vector.tensor_copy(out=pj0, in_=pjf[0:D, :])
    nc.vector.tensor_copy(out=pj1, in_=pjf[D:2 * D, :])

    bf1 = stage.tile([1, C], F32, tag="biasstage")
    nc.sync.dma_start(out=bf1, in_=bias8)
    nc.gpsimd.partition_broadcast(out_ap=bias_bc, in_ap=bf1)

    # ---- qkv projections ----
    # q^T, k^T: [d, t] = w[c, d]^T @ x^T[c, t]
    for pi, dst in ((0, qTb), (1, kTb)):
        for tch in range(NT // 512):
            ps = psum.tile([P, 512], F32, tag="mm")
            for k in range(KC):
                nc.tensor.matmul(
                    ps, wbf[:, pi, k, :], xbf[:, k, 512 * tch:512 * (tch + 1)],
                    start=(k == 0), stop=(k == KC - 1))
            nc.vector.tensor_copy(out=dst[:, 512 * tch:512 * (tch + 1)], in_=ps)
    # v natural: [t, d] = x^T[c, t]^T @ w[c, d]; interleave ones columns
    nc.vector.memset(v_sb[:, :, D:D + 1], 1.0)
    nc.vector.memset(v_sb[:, :, 2 * D + 1:2 * D + 2], 1.0)
    for tt in range(TT):
        ps = psum.tile([P, P], F32, tag="mm")
        for k in range(KC):
            nc.tensor.matmul(
                ps, xbf[:, k, P * tt:P * (tt + 1)], wbf[:, 2, k, :],
                start=(k == 0), stop=(k == KC - 1))
        nc.vector.tensor_copy(out=v_sb[:, tt, 0:D], in_=ps[:, 0:D])
        nc.vector.tensor_copy(out=v_sb[:, tt, D + 1:2 * D + 1], in_=ps[:, D:2 * D])

    # ---- attention, one (local head, batch) pair at a time ----
    for h in range(HL):
        for b in range(B):
            qs = qTb[D * h:D * (h + 1), T * b:T * (b + 1)]   # [64, 1024]
            ks = kTb[D * h:D * (h + 1), T * b:T * (b + 1)]
            pt = ppool.tile([P, SM, T], BF16, tag="pt")      # p^T, bf16

            for m in range(SM):
                tc0 = 512 * (m // 4)   # first computed t (chunk-aligned)
                for cidx in range(tc0 // 512, T // 512):
                    ps = psum.tile([P, 512], F32, tag="mm")
                    nc.tensor.matmul(
                        ps, ks[:, P * m:P * (m + 1)],
                        qs[:, 512 * cidx:512 * (cidx + 1)],
                        start=True, stop=True)
                    nc.scalar.activation(
                        out=pt[:, m, 512 * cidx:512 * (cidx + 1)], in_=ps,
                        func=Exp, scale=SCALE)
                if tc0 > 0:
                    nc.vector.memset(pt[:, m, 0:tc0], 0.0)
                # zero out p where t < s (below causal diagonal)
                wdt = P * (m + 1) - tc0
                nc.gpsimd.affine_select(
                    pt[:, m, tc0:tc0 + wdt], pt[:, m, tc0:tc0 + wdt],
                    compare_op=mybir.AluOpType.is_ge, fill=0.0,
                    base=tc0 - P * m, pattern=[[1, wdt]], channel_multiplier=-1)

            # (p @ v)^T plus ones-column -> column sums, accumulated over s-tiles
            oa = psum_oa.tile([D + 1, T], F32, tag="oa")
            for cidx in range(T // 512):
                ms = [m for m in range(SM) if P * m < 512 * (cidx + 1)]
                for j, m in enumerate(ms):
                    nc.tensor.matmul(
                        oa[:, 512 * cidx:512 * (cidx + 1)],
                        v_sb[:, SM * b + m, (D + 1) * h:(D + 1) * (h + 1)],
                        pt[:, m, 512 * cidx:512 * (cidx + 1)],
                        start=(j == 0), stop=(j == len(ms) - 1))

            sums = rpool.tile([1, T], F32, tag="sums")
            nc.vector.tensor_copy(out=sums, in_=oa[D:D + 1, :])
            rec = rpool.tile([1, T], F32, tag="rec")
            nc.vector.reciprocal(out=rec, in_=sums)
            recbc = rpool.tile([P, T], F32, tag="recbc")
            nc.gpsimd.partition_broadcast(out_ap=recbc, in_ap=rec)

            hoT = hoT0 if h == 0 else hoT1
            nc.vector.tensor_mul(
                hoT[:, T * b:T * (b + 1)], oa[0:D, :], recbc[0:D, :])

            for m in range(SM):
                v0 = P * m
                ws = wpool.tile([P, T], F32, tag="ws")
                nc.vector.tensor_mul(
                    ws[:, v0:], pt[:, m, v0:], recbc[:, v0:])
                nc.sync.dma_start(
                    out=weiT[h, b, v0:v0 + P, v0:], in_=ws[:, v0:])

    # ---- output projection (partial over this core's 128 head dims) ----
    for tt in range(TT):
        for cidx in range(C // 512):
            ps = psum.tile([P, 512], F32, tag="mm")
            nc.tensor.matmul(
                ps, hoT0[:, P * tt:P * (tt + 1)],
                pj0[:, 512 * cidx:512 * (cidx + 1)], start=True, stop=False)
            nc.tensor.matmul(
                ps, hoT1[:, P * tt:P * (tt + 1)],
                pj1[:, 512 * cidx:512 * (cidx + 1)], start=False, stop=True)
            ob = opool.tile([P, 512], F32, tag="ob")
            nc.vector.tensor_add(
                ob, ps, bias_bc[:, 512 * cidx:512 * (cidx + 1)])
            nc.sync.dma_start(
                out=partial[P * tt:P * (tt + 1), 512 * cidx:512 * (cidx + 1)],
                in_=ob)


def build():
    if "nc" in _CACHED:
        return _CACHED["nc"]
    nc = bacc.Bacc("TRN2", target_bir_lowering=False, debug=False,
                   num_devices=N_CORES)
    xT = nc.dram_tensor("xT", [C, NT], F32, kind="ExternalInput").ap()
    wq = nc.dram_tensor("wq", [C, HL * D], F32, kind="ExternalInput").ap()
    wk = nc.dram_tensor("wk", [C, HL * D], F32, kind="ExternalInput").ap()
    wv = nc.dram_tensor("wv", [C, HL * D], F32, kind="ExternalInput").ap()
    projT = nc.dram_tensor("projT", [HL * D, C], F32, kind="ExternalInput").ap()
    bias8 = nc.dram_tensor("bias8", [1, C], F32, kind="ExternalInput").ap()
    weiT = nc.dram_tensor("weiT", [HL, B, T, T], F32, kind="ExternalOutput").ap()
    partial = nc.dram_tensor("partial", [NT, C], F32, kind="ExternalOutput").ap()
    with tile.TileContext(nc) as tc:
        _body(tc, xT, wq, wk, wv, projT, bias8, weiT, partial)
    nc.compile()
    _CACHED["nc"] = nc
    return nc


def make_in_maps(x, wk_, wq_, wv_, proj_w, proj_b):
    x2d = np.ascontiguousarray(np.asarray(x, dtype=np.float32).reshape(NT, C))
    xT = np.ascontiguousarray(x2d.T)
    projT_full = np.ascontiguousarray(np.asarray(proj_w, dtype=np.float32).T)
    bias8 = (np.asarray(proj_b, dtype=np.float32) / N_CORES).reshape(1, C)
    wq_ = np.asarray(wq_, dtype=np.float32)
    wk_ = np.asarray(wk_, dtype=np.float32)
    wv_ = np.asarray(wv_, dtype=np.float32)
    in_maps = []
    for c in range(N_CORES):
        hs = slice(HL * c, HL * (c + 1))
        in_maps.append({
            "xT": xT,
            "wq": np.ascontiguousarray(
                np.concatenate(list(wq_[hs]), axis=1)),
            "wk": np.ascontiguousarray(
                np.concatenate(list(wk_[hs]), axis=1)),
            "wv": np.ascontiguousarray(
                np.concatenate(list(wv_[hs]), axis=1)),
            "projT": np.ascontiguousarray(projT_full[P * c:P * (c + 1), :]),
            "bias8": bias8,
        })
    return in_maps


def assemble(results):
    wei = np.empty((H, B, T, T), dtype=np.float32)
    for c, r in enumerate(results):
        wei[HL * c:HL * (c + 1)] = np.swapaxes(r["weiT"], -1, -2)
    out = np.zeros((NT, C), dtype=np.float32)
    for r in results:
        out += r["partial"]
    return wei, out.reshape(B, T, C)


def kernel(x, wk, wq, wv, proj_w, proj_b, _run_kwargs=None):
    nc = build()
    in_maps = make_in_maps(x, wk, wq, wv, proj_w, proj_b)
    kw = dict(_run_kwargs or {})
    res = bass_utils.run_bass_kernel_spmd(
        nc, in_maps, core_ids=list(range(N_CORES)), **kw)
    _CACHED["last_results"] = res
    return assemble(res.results)


# revision 7
# speedup vs baseline: 1.2930x; 1.2930x over previous
"""Bass/Trainium2 kernel for DecodeMultiHeadAttention (16 heads, B=2, T=1024, C=1024).

Sharding: tensor-parallel over heads -- 2 heads per core x 8 cores.

Math notes (vs the jax reference):
  * The ALiBi bias is base**clip(j-i,0) which is exactly 1.0 on every causal
    (unmasked) position, and softmax is shift-invariant, so it drops out.
  * Scores are tiny (std ~0.1) so softmax needs no max-subtraction:
    wei = exp(s*scale) / sum(exp(s*scale)) over the causal extent.

Per core (2 local heads h in {0,1}, batch b in {0,1}):
  * qT,kT = [d,t]-layout projections (PE contracts embedding dim c, using a
    host-transposed xT input so all DMA is contiguous).
  * v in natural [t,d] layout with a ones-column appended, so a single PE
    matmul per pair produces both (p @ v)^T and the softmax denominator.
  * Scores are computed transposed, sT[s,t] = k_s . q_t, so the p @ v matmul
    needs no on-chip transpose. wei is written transposed ([s,t]) and the
    host swaps the last two axes while unsharding.
  * Only the causal (lower-triangle in [t,s] == upper in [s,t]) block rows are
    computed and written; the rest of the wei output buffer stays zero
    (outputs are zero-initialized).
  * Output projection: each core computes a partial product over its 128 head
    dims (+ bias/8); the host sums the 8 partials.
"""

import numpy as np

import concourse.bacc as bacc
import concourse.tile as tile
from concourse import mybir
from concourse import bass_utils

N_CORES = 8
B, T, C = 2, 1024, 1024
H, D = 16, 64          # total heads, head dim
HL = H // N_CORES      # heads per core (2)
NT = B * T             # 2048 rows total
P = 128
KC = C // P            # 8 contraction tiles over embedding dim
TT = NT // P           # 16 row tiles
SM = T // P            # 8 s-tiles per (head, batch) pair
SCALE = float(C) ** -0.5
F32 = mybir.dt.float32
BF16 = mybir.dt.bfloat16

_CACHED = {}


def _body(tc, xT, wq, wk, wv, projT, bias8, weiT, partial, ctx, pfx=""):
    nc = tc.nc
    Exp = mybir.ActivationFunctionType.Exp

    const = ctx.enter_context(tc.tile_pool(name=pfx + "const", bufs=1))
    stage = ctx.enter_context(tc.tile_pool(name=pfx + "stage", bufs=2))
    ppool = ctx.enter_context(tc.tile_pool(name=pfx + "ppool", bufs=2))
    wpool = ctx.enter_context(tc.tile_pool(name=pfx + "wpool", bufs=4))
    rpool = ctx.enter_context(tc.tile_pool(name=pfx + "rpool", bufs=2))
    opool = ctx.enter_context(tc.tile_pool(name=pfx + "opool", bufs=4))
    psum = ctx.enter_context(
        tc.tile_pool(name=pfx + "psum", bufs=3, space="PSUM"))
    psum_oa = ctx.enter_context(
        tc.tile_pool(name=pfx + "psum_oa", bufs=2, space="PSUM"))

    # ---- persistent SBUF tensors ----
    xbf = const.tile([P, KC, NT], BF16)       # x^T, bf16: [c-part, c-tile, t]
    wbf = const.tile([P, 3, KC, P], BF16)     # q/k/v weights: [c-part, proj, c-tile, d]
    qTb = const.tile([P, NT], BF16)           # q^T: [2 heads x 64 d, t]
    kTb = const.tile([P, NT], BF16)
    v_sb = const.tile([P, TT, 2 * (D + 1)], BF16)  # v natural + ones cols
    pj0 = const.tile([D, C], BF16)            # projT rows 0:64   (local head 0)
    pj1 = const.tile([D, C], BF16)            # projT rows 64:128 (local head 1)
    hoT0 = const.tile([D, NT], BF16)          # head-out^T, local head 0
    hoT1 = const.tile([D, NT], BF16)
    bias_bc = const.tile([P, C], F32)         # bias/8 broadcast to 128 partitions

    # ---- load + cast inputs ----
    for k in range(KC):
        xf = stage.tile([P, NT], F32, tag="xstage")
        nc.sync.dma_start(out=xf, in_=xT[k * P:(k + 1) * P, :])
        nc.scalar.copy(out=xbf[:, k, :], in_=xf)

    for i, w in enumerate([wq, wk, wv]):
        wf = stage.tile([P, KC, P], F32, tag="wstage")
        nc.sync.dma_start(out=wf, in_=w.rearrange("(k p) d -> p k d", p=P))
        nc.vector.tensor_copy(out=wbf[:, i], in_=wf)

    pjf = stage.tile([P, C], F32, tag="pjstage")
    nc.sync.dma_start(out=pjf, in_=projT)
    nc.vector.tensor_copy(out=pj0, in_=pjf[0:D, :])
    nc.vector.tensor_copy(out=pj1, in_=pjf[D:2 * D, :])

    bf1 = stage.tile([1, C], F32, tag="biasstage")
    nc.sync.dma_start(out=bf1, in_=bias8)
    nc.gpsimd.partition_broadcast(out_ap=bias_bc, in_ap=bf1)

    # ---- qkv projections ----
    # q^T, k^T: [d, t] = w[c, d]^T @ x^T[c, t]
    for pi, dst in ((0, qTb), (1, kTb)):
        for tch in range(NT // 512):
            ps = psum.tile([P, 512], F32, tag="mm")
            for k in range(KC):
                nc.tensor.matmul(
                    ps, wbf[:, pi, k, :], xbf[:, k, 512 * tch:512 * (tch + 1)],
                    start=(k == 0), stop=(k == KC - 1))
            nc.vector.tensor_copy(out=dst[:, 512 * tch:512 * (tch + 1)], in_=ps)
    # v natural: [t, d] = x^T[c, t]^T @ w[c, d]; interleave ones columns
    nc.vector.memset(v_sb[:, :, D:D + 1], 1.0)
    nc.vector.memset(v_sb[:, :, 2 * D + 1:2 * D + 2], 1.0)
    for tt in range(TT):
        ps = psum.tile([P, P], F32, tag="mm")
        for k in range(KC):
            nc.tensor.matmul(
                ps, xbf[:, k, P * tt:P * (tt + 1)], wbf[:, 2, k, :],
                start=(k == 0), stop=(k == KC - 1))
        nc.vector.tensor_copy(out=v_sb[:, tt, 0:D], in_=ps[:, 0:D])
        nc.vector.tensor_copy(out=v_sb[:, tt, D + 1:2 * D + 1], in_=ps[:, D:2 * D])

    # ---- attention, one (local head, batch) pair at a time ----
    for h in range(HL):
        for b in range(B):
            qs = qTb[D * h:D * (h + 1), T * b:T * (b + 1)]   # [64, 1024]
            ks = kTb[D * h:D * (h + 1), T * b:T * (b + 1)]
            pt = ppool.tile([P, SM, T], BF16, tag="pt")      # p^T, bf16

            for m in range(SM):
                tc0 = 512 * (m // 4)   # first computed t (chunk-aligned)
                for cidx in range(tc0 // 512, T // 512):
                    ps = psum.tile([P, 512], F32, tag="mm")
                    nc.tensor.matmul(
                        ps, ks[:, P * m:P * (m + 1)],
                        qs[:, 512 * cidx:512 * (cidx + 1)],
                        start=True, stop=True)
                    nc.scalar.activation(
                        out=pt[:, m, 512 * cidx:512 * (cidx + 1)], in_=ps,
                        func=Exp, scale=SCALE)
                if tc0 > 0:
                    nc.vector.memset(pt[:, m, 0:tc0], 0.0)
                # zero out p where t < s (below causal diagonal)
                wdt = P * (m + 1) - tc0
                nc.gpsimd.affine_select(
                    pt[:, m, tc0:tc0 + wdt], pt[:, m, tc0:tc0 + wdt],
                    compare_op=mybir.AluOpType.is_ge, fill=0.0,
                    base=tc0 - P * m, pattern=[[1, wdt]], channel_multiplier=-1)

            # (p @ v)^T plus ones-column -> column sums, accumulated over s-tiles
            oa = psum_oa.tile([D + 1, T], F32, tag="oa")
            for cidx in range(T // 512):
                ms = [m for m in range(SM) if P * m < 512 * (cidx + 1)]
                for j, m in enumerate(ms):
                    nc.tensor.matmul(
                        oa[:, 512 * cidx:512 * (cidx + 1)],
                        v_sb[:, SM * b + m, (D + 1) * h:(D + 1) * (h + 1)],
                        pt[:, m, 512 * cidx:512 * (cidx + 1)],
                        start=(j == 0), stop=(j == len(ms) - 1))

            sums = rpool.tile([1, T], F32, tag="sums")
            nc.vector.tensor_copy(out=sums, in_=oa[D:D + 1, :])
            rec = rpool.tile([1, T], F32, tag="rec")
            nc.vector.reciprocal(out=rec, in_=sums)
            recbc = rpool.tile([P, T], F32, tag="recbc")
            nc.gpsimd.partition_broadcast(out_ap=recbc, in_ap=rec)

            hoT = hoT0 if h == 0 else hoT1
            nc.vector.tensor_mul(
                hoT[:, T * b:T * (b + 1)], oa[0:D, :], recbc[0:D, :])

            for m in range(SM):
                v0 = P * m
                ws = wpool.tile([P, T], F32, tag="ws")
                nc.vector.tensor_mul(
                    ws[:, v0:], pt[:, m, v0:], recbc[:, v0:])
                nc.sync.dma_start(
                    out=weiT[h, b, v0:v0 + P, v0:], in_=ws[:, v0:])

    # ---- output projection (partial over this core's 128 head dims) ----
    for tt in range(TT):
        for cidx in range(C // 512):
            ps = psum.tile([P, 512], F32, tag="mm")
            nc.tensor.matmul(
                ps, hoT0[:, P * tt:P * (tt + 1)],
                pj0[:, 512 * cidx:512 * (cidx + 1)], start=True, stop=False)
            nc.tensor.matmul(
                ps, hoT1[:, P * tt:P * (tt + 1)],
                pj1[:, 512 * cidx:512 * (cidx + 1)], start=False, stop=True)
            ob = opool.tile([P, 512], F32, tag="ob")
            nc.vector.tensor_add(
                ob, ps, bias_bc[:, 512 * cidx:512 * (cidx + 1)])
            nc.sync.dma_start(
                out=partial[P * tt:P * (tt + 1), 512 * cidx:512 * (cidx + 1)],
                in_=ob)


def build(reps=1):
    key = ("nc", reps)
    if key in _CACHED:
        return _CACHED[key]
    nc = bacc.Bacc("TRN2", target_bir_lowering=False, debug=False,
                   num_devices=N_CORES)
    xT = nc.dram_tensor("xT", [C, NT], F32, kind="ExternalInput").ap()
    wq = nc.dram_tensor("wq", [C, HL * D], F32, kind="ExternalInput").ap()
    wk = nc.dram_tensor("wk", [C, HL * D], F32, kind="ExternalInput").ap()
    wv = nc.dram_tensor("wv", [C, HL * D], F32, kind="ExternalInput").ap()
    projT = nc.dram_tensor("projT", [HL * D, C], F32, kind="ExternalInput").ap()
    bias8 = nc.dram_tensor("bias8", [1, C], F32, kind="ExternalInput").ap()
    weiT = nc.dram_tensor("weiT", [HL, B, T, T], F32, kind="ExternalOutput").ap()
    partial = nc.dram_tensor("partial", [NT, C], F32, kind="ExternalOutput").ap()
    from contextlib import ExitStack
    with tile.TileContext(nc) as tc:
        for r in range(reps):
            with ExitStack() as ctx:
                _body(tc, xT, wq, wk, wv, projT, bias8, weiT, partial, ctx,
                      pfx=f"r{r}_" if reps > 1 else "")
    nc.compile()
    _CACHED[key] = nc
    return nc


def make_in_maps(x, wk, wq, wv, proj_w, proj_b):
    x2d = np.ascontiguousarray(np.asarray(x, dtype=np.float32).reshape(NT, C))
    xT = np.ascontiguousarray(x2d.T)
    projT_full = np.ascontiguousarray(np.asarray(proj_w, dtype=np.float32).T)
    bias8 = (np.asarray(proj_b, dtype=np.float32) / N_CORES).reshape(1, C)
    wq_ = np.asarray(wq, dtype=np.float32)
    wk_ = np.asarray(wk, dtype=np.float32)
    wv_ = np.asarray(wv, dtype=np.float32)
    in_maps = []
    for c in range(N_CORES):
        hs = slice(HL * c, HL * (c + 1))
        in_maps.append({
            "xT": xT,
            "wq": np.ascontiguousarray(
                np.concatenate(list(wq_[hs]), axis=1)),
            "wk": np.ascontiguousarray(
                np.concatenate(list(wk_[hs]), axis=1)),
            "wv": np.ascontiguousarray(
                np.concatenate(list(wv_[hs]), axis=1)),
            "projT": np.ascontiguousarray(projT_full[P * c:P * (c + 1), :]),
            "bias8": bias8,
        })
    return in_maps


def assemble(results):
    wei = np.empty((H, B, T, T), dtype=np.float32)
    for c, r in enumerate(results):
        wei[HL * c:HL * (c + 1)] = np.swapaxes(r["weiT"], -1, -2)
    out = np.zeros((NT, C), dtype=np.float32)
    for r in results:
        out += r["partial"]
    return wei, out.reshape(B, T, C)


def kernel(x, wk, wq, wv, proj_w, proj_b, _run_kwargs=None):
    nc = build()
    in_maps = make_in_maps(x, wk, wq, wv, proj_w, proj_b)
    kw = dict(_run_kwargs or {})
    res = bass_utils.run_bass_kernel_spmd(
        nc, in_maps, core_ids=list(range(N_CORES)), **kw)
    _CACHED["last_results"] = res
    return assemble(res.results)
